# revision 1
# baseline (speedup 1.0000x reference)
"""ChebGCN (K=2, 3 layers) Trainium2 kernel — 8-core SPMD.

Sharding: nodes are split across 8 cores (12500/core, padded to 12544 for
128 alignment). Edges are bucketed by destination core, sorted by
destination node and packed into 128-edge chunks aligned to 128-node
destination tiles; chunk counts per tile are equalized across cores so all
8 cores run one SPMD program. Per propagate, each core gathers the 128
source rows of a chunk with one indirect DMA (one row index per partition),
builds a weighted one-hot on DVE in one fused tensor_scalar
((iota == dest_pos) * w) and the TensorEngine accumulates feat^T @ onehot
into PSUM, producing segment sums in transposed layout (features on
partitions, nodes on free dim). Dense 64-wide weight matmuls + bias/relu
stay in transposed layout; per 128-node tile the result is PE-transposed
back to row-major and DMA'd to DRAM, and an 8-core AllGather rebuilds the
full gather table for the next layer.
"""

import sys

for _p in ("/opt/trn_rl_repo",):
    if _p not in sys.path:
        sys.path.insert(0, _p)

import math
import time
from contextlib import ExitStack

import numpy as np

import concourse.bacc as bacc
import concourse.bass as bass
import concourse.mybir as mybir
import concourse.tile as tile
from concourse.bass_utils import run_bass_kernel_spmd

F32 = mybir.dt.float32
I32 = mybir.dt.int32

M_CORES = 8
MG = 64  # chunks per metadata (pos/w/idx) load
LAST_TIMES = []  # wall times of repeat runs (filled by run(timeit=N))


# ---------------------------------------------------------------- host prep
def host_prep(adj, n_nodes, npc, npcp):
    """Bucket/sort/pad edges -> per-core slot arrays + shared chunk schedule.

    Returns (sched, per_core): sched[j] = (tile_idx, is_first, is_last) per
    128-edge chunk (identical across cores); per_core[c] has offs (int32),
    pos, wgt, each [128, T].
    """
    n_tiles = npcp // 128
    row = adj[0].astype(np.int64)
    col = adj[1].astype(np.int64)

    deg = np.bincount(row, minlength=n_nodes).astype(np.float64)
    dis = np.where(deg > 0, 1.0 / np.sqrt(np.maximum(deg, 1)), 0.0).astype(
        np.float32
    )
    w_all = (-(dis[row] * dis[col])).astype(np.float32)
    colp = (col // npc) * npcp + (col % npc)

    core_of = row // npc
    per_core_raw = []
    counts = np.zeros((M_CORES, n_tiles), dtype=np.int64)
    for c in range(M_CORES):
        sel = np.nonzero(core_of == c)[0]
        r_loc = row[sel] - c * npc
        order = np.argsort(r_loc, kind="stable")
        sel = sel[order]
        per_core_raw.append((r_loc[order], colp[sel], w_all[sel]))
        counts[c] = np.bincount(r_loc[order] // 128, minlength=n_tiles)

    nch = np.maximum(np.ceil(counts / 128.0).astype(np.int64).max(axis=0), 1)
    t_chunks = int(nch.sum())

    sched = []
    for t in range(n_tiles):
        for k in range(int(nch[t])):
            sched.append((t, k == 0, k == int(nch[t]) - 1))
    tile_base = np.concatenate([[0], np.cumsum(nch)[:-1]]) * 128

    per_core = []
    for c in range(M_CORES):
        r_loc, cp, wc = per_core_raw[c]
        offs = np.zeros(t_chunks * 128, dtype=np.int32)
        pos = np.zeros(t_chunks * 128, dtype=np.float32)
        wgt = np.zeros(t_chunks * 128, dtype=np.float32)
        t_of = r_loc // 128
        cnt = np.bincount(t_of, minlength=n_tiles)
        idx_within = np.zeros_like(r_loc)
        start = 0
        for t in range(n_tiles):
            e = start + int(cnt[t])
            idx_within[start:e] = np.arange(e - start)
            start = e
        slots = tile_base[t_of] + idx_within
        offs[slots] = cp
        pos[slots] = (r_loc - t_of * 128).astype(np.float32)
        wgt[slots] = wc
        per_core.append(
            dict(
                offs=np.ascontiguousarray(offs.reshape(t_chunks, 128).T),
                pos=np.ascontiguousarray(pos.reshape(t_chunks, 128).T),
                wgt=np.ascontiguousarray(wgt.reshape(t_chunks, 128).T),
            )
        )
    return sched, per_core


# ------------------------------------------------------------- bass program
def build_program(sched, npcp, fin, fhid, fout, feat_bufs=10):
    n_tiles = npcp // 128
    np_all = npcp * M_CORES
    t_chunks = len(sched)

    nc = bacc.Bacc(
        "TRN2",
        target_bir_lowering=False,
        debug=False,
        enable_asserts=True,
        num_devices=M_CORES,
    )

    x_tab = nc.dram_tensor("x_tab", [np_all, fin], F32, kind="ExternalInput")
    # layer-1 source features pre-gathered on host, partition-major:
    # feat1[p, j*fin:(j+1)*fin] = x[col of edge slot (chunk j, lane p)]
    feat1_d = nc.dram_tensor("feat1", [128, t_chunks * fin], F32,
                             kind="ExternalInput")
    xT_d = nc.dram_tensor("xT", [fin, npcp], F32, kind="ExternalInput")
    offs_d = nc.dram_tensor("offs", [128, t_chunks], I32,
                            kind="ExternalInput")
    pos_d = nc.dram_tensor("pos", [128, t_chunks], F32, kind="ExternalInput")
    wgt_d = nc.dram_tensor("wgt", [128, t_chunks], F32, kind="ExternalInput")
    w10 = nc.dram_tensor("w10", [fin, fhid], F32, kind="ExternalInput")
    w11 = nc.dram_tensor("w11", [fin, fhid], F32, kind="ExternalInput")
    wx0 = nc.dram_tensor("wx0", [fhid, fhid], F32, kind="ExternalInput")
    wx1 = nc.dram_tensor("wx1", [fhid, fhid], F32, kind="ExternalInput")
    w20 = nc.dram_tensor("w20", [fhid, fout], F32, kind="ExternalInput")
    w21 = nc.dram_tensor("w21", [fhid, fout], F32, kind="ExternalInput")
    b1_d = nc.dram_tensor("b1", [fhid, 1], F32, kind="ExternalInput")
    bx_d = nc.dram_tensor("bx", [fhid, 1], F32, kind="ExternalInput")
    b2_d = nc.dram_tensor("b2", [fout, 1], F32, kind="ExternalInput")
    iota_d = nc.dram_tensor("iota", [128, 128], F32, kind="ExternalInput")
    ident_d = nc.dram_tensor("ident", [fhid, fhid], F32, kind="ExternalInput")
    out_d = nc.dram_tensor("out", [fout, npcp], F32, kind="ExternalOutput")

    hT1_d = nc.dram_tensor("hT1", [fhid, npcp], F32)
    hT2_d = nc.dram_tensor("hT2", [fhid, npcp], F32)
    rows1 = nc.dram_tensor("rows1", [npcp, fhid], F32)
    rows2 = nc.dram_tensor("rows2", [npcp, fhid], F32)
    tab2 = nc.dram_tensor("tab2", [np_all, fhid], F32, addr_space="Shared")
    tab3 = nc.dram_tensor("tab3", [np_all, fhid], F32, addr_space="Shared")

    rg = [list(range(M_CORES))]

    with ExitStack() as ctx:
        tc = ctx.enter_context(tile.TileContext(nc))
        const = ctx.enter_context(tc.tile_pool(name="const", bufs=1))
        fsgp = ctx.enter_context(tc.tile_pool(name="fsgp", bufs=3))
        featp = ctx.enter_context(tc.tile_pool(name="featp", bufs=feat_bufs))
        meta = ctx.enter_context(tc.tile_pool(name="meta", bufs=3))
        ohp = ctx.enter_context(tc.tile_pool(name="ohp", bufs=4))
        txp = ctx.enter_context(tc.tile_pool(name="txp", bufs=3))
        rhsp = ctx.enter_context(tc.tile_pool(name="rhsp", bufs=3))
        otp = ctx.enter_context(tc.tile_pool(name="otp", bufs=3))
        stg = ctx.enter_context(tc.tile_pool(name="stg", bufs=3))
        psA = ctx.enter_context(tc.tile_pool(name="psA", bufs=3, space="PSUM"))
        psB = ctx.enter_context(tc.tile_pool(name="psB", bufs=2, space="PSUM"))
        psT = ctx.enter_context(tc.tile_pool(name="psT", bufs=2, space="PSUM"))

        def load_const(dram, shape, name, dtype=F32):
            t = const.tile(shape, dtype, tag=name)
            nc.sync.dma_start(out=t[:], in_=dram[:, :])
            return t

        iota_t = load_const(iota_d, [128, 128], "iota")
        ident_t = load_const(ident_d, [fhid, fhid], "ident")
        w10_t = load_const(w10, [fin, fhid], "w10")
        w11_t = load_const(w11, [fin, fhid], "w11")
        wx0_t = load_const(wx0, [fhid, fhid], "wx0")
        wx1_t = load_const(wx1, [fhid, fhid], "wx1")
        w20_t = load_const(w20, [fhid, fout], "w20")
        w21_t = load_const(w21, [fhid, fout], "w21")
        b1_t = load_const(b1_d, [fhid, 1], "b1")
        bx_t = load_const(bx_d, [fhid, 1], "bx")
        b2_t = load_const(b2_d, [fout, 1], "b2")

        layers = [
            dict(table=x_tab, rhs_d=xT_d, W0=w10_t, W1=w11_t, b=b1_t,
                 relu=True, fo=fhid, hT_next=hT1_d, rows=rows1,
                 tab_next=tab2),
            dict(table=tab2, rhs_d=hT1_d, W0=wx0_t, W1=wx1_t, b=bx_t,
                 relu=True, fo=fhid, hT_next=hT2_d, rows=rows2,
                 tab_next=tab3),
            dict(table=tab3, rhs_d=hT2_d, W0=w20_t, W1=w21_t, b=b2_t,
                 relu=False, fo=fout, hT_next=None, rows=None,
                 tab_next=None),
        ]

        for li, L in enumerate(layers):
            fo = L["fo"]
            pos_t = w_t = offs_t = fsg = None
            for j, (t, first, last) in enumerate(sched):
                if j % MG == 0:
                    mw = min(MG, t_chunks - j)
                    pos_t = meta.tile([128, MG], F32, tag="pos")
                    nc.sync.dma_start(out=pos_t[:, :mw],
                                      in_=pos_d[:, j:j + mw])
                    w_t = meta.tile([128, MG], F32, tag="wgt")
                    nc.sync.dma_start(out=w_t[:, :mw],
                                      in_=wgt_d[:, j:j + mw])
                    if li > 0:
                        offs_t = meta.tile([128, MG], I32, tag="offs")
                        nc.sync.dma_start(out=offs_t[:, :mw],
                                          in_=offs_d[:, j:j + mw])
                    else:
                        fsg = fsgp.tile([128, MG * fin], F32, tag="fsg")
                        nc.sync.dma_start(
                            out=fsg[:, :mw * fin],
                            in_=feat1_d[:, j * fin:(j + mw) * fin],
                        )
                jm = j % MG
                if li > 0:
                    ft = featp.tile([128, fin], F32, tag="fb")
                    nc.gpsimd.indirect_dma_start(
                        out=ft[:],
                        out_offset=None,
                        in_=L["table"][:, :],
                        in_offset=bass.IndirectOffsetOnAxis(
                            ap=offs_t[:, jm:jm + 1], axis=0
                        ),
                    )
                    lhs_ap = ft[:]
                else:
                    lhs_ap = fsg[:, jm * fin:(jm + 1) * fin]
                oh = ohp.tile([128, 128], F32, tag="oh")
                nc.vector.tensor_scalar(
                    out=oh[:],
                    in0=iota_t[:],
                    scalar1=pos_t[:, jm:jm + 1],
                    scalar2=w_t[:, jm:jm + 1],
                    op0=mybir.AluOpType.is_equal,
                    op1=mybir.AluOpType.mult,
                )
                if first:
                    pa = psA.tile([fhid, 128], F32, tag="pa")
                nc.tensor.matmul(
                    pa[:], lhsT=lhs_ap, rhs=oh[:], start=first, stop=last
                )
                if last:
                    txT = txp.tile([fhid, 128], F32, tag="tx")
                    nc.scalar.activation(
                        txT[:], pa[:], mybir.ActivationFunctionType.Copy
                    )
                    rhs_t = rhsp.tile([fin, 128], F32, tag="rhs")
                    nc.sync.dma_start(
                        out=rhs_t[:],
                        in_=L["rhs_d"][:, t * 128:(t + 1) * 128],
                    )
                    pb = psB.tile([fo, 128], F32, tag="pb")
                    nc.tensor.matmul(pb[:], lhsT=L["W0"][:], rhs=rhs_t[:],
                                     start=True, stop=False)
                    nc.tensor.matmul(pb[:], lhsT=L["W1"][:], rhs=txT[:],
                                     start=False, stop=True)
                    ot = otp.tile([fo, 128], F32, tag="ot")
                    nc.scalar.activation(
                        ot[:],
                        pb[:],
                        mybir.ActivationFunctionType.Relu
                        if L["relu"]
                        else mybir.ActivationFunctionType.Identity,
                        bias=L["b"][:],
                    )
                    if L["hT_next"] is not None:
                        nc.sync.dma_start(
                            out=L["hT_next"][:, t * 128:(t + 1) * 128],
                            in_=ot[:],
                        )
                        pt = psT.tile([128, fhid], F32, tag="pt")
                        nc.tensor.transpose(
                            out=pt[:], in_=ot[:], identity=ident_t[:]
                        )
                        st = stg.tile([128, fhid], F32, tag="st")
                        nc.scalar.activation(
                            st[:], pt[:], mybir.ActivationFunctionType.Copy
                        )
                        nc.sync.dma_start(
                            out=L["rows"][t * 128:(t + 1) * 128, :],
                            in_=st[:],
                        )
                    else:
                        nc.sync.dma_start(
                            out=out_d[:, t * 128:(t + 1) * 128], in_=ot[:]
                        )
            if L["tab_next"] is not None:
                nc.gpsimd.collective_compute(
                    "AllGather",
                    mybir.AluOpType.bypass,
                    replica_groups=rg,
                    ins=[L["rows"][:, :]],
                    outs=[L["tab_next"][:, :]],
                )

    nc.compile()
    return nc


# ------------------------------------------------------------------ runner
def make_in_maps(inputs, n_nodes, npc, npcp, fin, fhid, fout, per_core):
    np_all = npcp * M_CORES
    x = np.asarray(inputs["x"], dtype=np.float32)
    x_tab = np.zeros((np_all, fin), dtype=np.float32)
    for c in range(M_CORES):
        x_tab[c * npcp:c * npcp + npc] = x[c * npc:(c + 1) * npc]
    iota = np.broadcast_to(
        np.arange(128, dtype=np.float32), (128, 128)
    ).copy()
    common = dict(
        x_tab=x_tab,
        w10=np.asarray(inputs["W1_0"], np.float32),
        w11=np.asarray(inputs["W1_1"], np.float32),
        wx0=np.asarray(inputs["Wx_0"], np.float32),
        wx1=np.asarray(inputs["Wx_1"], np.float32),
        w20=np.asarray(inputs["W2_0"], np.float32),
        w21=np.asarray(inputs["W2_1"], np.float32),
        b1=np.asarray(inputs["b1"], np.float32).reshape(fhid, 1),
        bx=np.asarray(inputs["bx"], np.float32).reshape(fhid, 1),
        b2=np.asarray(inputs["b2"], np.float32).reshape(fout, 1),
        iota=iota,
        ident=np.eye(fhid, dtype=np.float32),
    )
    in_maps = []
    for c in range(M_CORES):
        xT_c = np.zeros((fin, npcp), dtype=np.float32)
        xT_c[:, :npc] = x[c * npc:(c + 1) * npc].T
        offs = per_core[c]["offs"]
        feat1 = np.ascontiguousarray(
            x_tab[offs].reshape(128, offs.shape[1] * fin)
        )
        in_maps.append(dict(common, xT=xT_c, feat1=feat1, **per_core[c]))
    return in_maps


def run(inputs, n_nodes, fin, fhid, fout, trace=False, trace_kwargs=None,
        timeit=0):
    npc = n_nodes // M_CORES
    npcp = int(math.ceil(npc / 128.0)) * 128

    adj = np.asarray(inputs["adj"], dtype=np.int32)
    sched, per_core = host_prep(adj, n_nodes, npc, npcp)
    nc = build_program(sched, npcp, fin, fhid, fout)
    in_maps = make_in_maps(
        inputs, n_nodes, npc, npcp, fin, fhid, fout, per_core
    )
    res = run_bass_kernel_spmd(
        nc,
        in_maps,
        core_ids=list(range(M_CORES)),
        trace=trace,
        **(trace_kwargs or {}),
    )
    times = []
    for _ in range(timeit):
        t0 = time.perf_counter()
        run_bass_kernel_spmd(nc, in_maps, core_ids=list(range(M_CORES)))
        times.append(time.perf_counter() - t0)
    if times:
        print("repeat wall times (s):", [f"{t:.3f}" for t in times])
        global LAST_TIMES
        LAST_TIMES = times
    out = np.concatenate(
        [res.results[c]["out"][:, :npc].T for c in range(M_CORES)], axis=0
    )
    return out, res


def kernel(**inputs):
    out, _ = run(inputs, n_nodes=100000, fin=64, fhid=64, fout=16)
    return out



# revision 7
# speedup vs baseline: 7.1790x; 7.1790x over previous
"""ChebGCN (K=2, 3 layers) Trainium2 kernel — 8-core SPMD.

Sharding: nodes are split across 8 cores (12500/core, padded to 12544 for
128 alignment). Edges are bucketed by destination core, sorted by
destination node and packed into 128-edge chunks aligned to 128-node
destination tiles; chunk counts per tile are equalized across cores so all
8 cores run one SPMD program.

Host->device traffic is minimized (the PJRT/axon tunnel is ~50 MB/s and
dominates wall time): each core receives only its bf16 x slice (1.6 MB),
one packed int32 per edge slot (source_index*128 + dest_pos) and a bf16
edge weight. The full gather table is built on device by an 8-core
AllGather of the x slices; layer-1 dense-term inputs (x^T tiles) are
produced on device by PE transposes. Per propagate, each core gathers the
128 source rows of a chunk with one indirect DMA, builds a weighted
one-hot on DVE ((iota == dest_pos) * w) and the TensorEngine accumulates
feat^T @ onehot into PSUM, producing segment sums in transposed layout.
Dense 64-wide weight matmuls + bias/relu stay in transposed layout with
the running feature map resident in SBUF; per 128-node tile the result is
PE-transposed back to row-major, DMA'd to DRAM and AllGathered to rebuild
the gather table for the next layer. Output leaves the device as bf16.
"""

import sys

for _p in ("/opt/trn_rl_repo",):
    if _p not in sys.path:
        sys.path.insert(0, _p)

import math
import time
from contextlib import ExitStack

import ml_dtypes
import numpy as np

import concourse.bacc as bacc
import concourse.bass as bass
import concourse.mybir as mybir
import concourse.tile as tile
from concourse.bass_utils import run_bass_kernel_spmd

F32 = mybir.dt.float32
I32 = mybir.dt.int32
BF16 = mybir.dt.bfloat16
NP_BF16 = ml_dtypes.bfloat16

M_CORES = 8
MG = 64  # chunks per metadata (packed/wgt) load
LAST_TIMES = []  # wall times of repeat runs (filled by run(timeit=N))


# ---------------------------------------------------------------- host prep
def host_prep(adj, n_nodes, npc, npcp):
    """Bucket/sort/pad edges -> per-core slot arrays + shared chunk schedule.

    Returns (sched, per_core): sched[j] = (tile_idx, is_first, is_last) per
    128-edge chunk (identical across cores); per_core[c] has
    packed = src_padded_index*128 + dest_pos (int32) and wgt (bf16),
    each [128, T].
    """
    n_tiles = npcp // 128
    row = adj[0].astype(np.int64)
    col = adj[1].astype(np.int64)

    deg = np.bincount(row, minlength=n_nodes).astype(np.float64)
    dis = np.where(deg > 0, 1.0 / np.sqrt(np.maximum(deg, 1)), 0.0).astype(
        np.float32
    )
    w_all = (-(dis[row] * dis[col])).astype(np.float32)
    colp = (col // npc) * npcp + (col % npc)

    core_of = row // npc
    per_core_raw = []
    counts = np.zeros((M_CORES, n_tiles), dtype=np.int64)
    for c in range(M_CORES):
        sel = np.nonzero(core_of == c)[0]
        r_loc = row[sel] - c * npc
        order = np.argsort(r_loc, kind="stable")
        sel = sel[order]
        per_core_raw.append((r_loc[order], colp[sel], w_all[sel]))
        counts[c] = np.bincount(r_loc[order] // 128, minlength=n_tiles)

    nch = np.maximum(np.ceil(counts / 128.0).astype(np.int64).max(axis=0), 1)
    t_chunks = int(nch.sum())

    sched = []
    for t in range(n_tiles):
        for k in range(int(nch[t])):
            sched.append((t, k == 0, k == int(nch[t]) - 1))
    tile_base = np.concatenate([[0], np.cumsum(nch)[:-1]]) * 128

    per_core = []
    for c in range(M_CORES):
        r_loc, cp, wc = per_core_raw[c]
        packed = np.zeros(t_chunks * 128, dtype=np.int32)
        wgt = np.zeros(t_chunks * 128, dtype=np.float32)
        t_of = r_loc // 128
        cnt = np.bincount(t_of, minlength=n_tiles)
        idx_within = np.zeros_like(r_loc)
        start = 0
        for t in range(n_tiles):
            e = start + int(cnt[t])
            idx_within[start:e] = np.arange(e - start)
            start = e
        slots = tile_base[t_of] + idx_within
        packed[slots] = (cp * 128 + (r_loc - t_of * 128)).astype(np.int32)
        wgt[slots] = wc
        per_core.append(
            dict(
                packed=np.ascontiguousarray(
                    packed.reshape(t_chunks, 128).T
                ),
                wgt=np.ascontiguousarray(
                    wgt.reshape(t_chunks, 128).T.astype(NP_BF16)
                ),
            )
        )
    return sched, per_core


# ------------------------------------------------------------- bass program
def build_program(sched, npcp, fin, fhid, fout):
    n_tiles = npcp // 128
    np_all = npcp * M_CORES
    t_chunks = len(sched)

    nc = bacc.Bacc(
        "TRN2",
        target_bir_lowering=False,
        debug=False,
        enable_asserts=True,
        num_devices=M_CORES,
    )

    x_loc_d = nc.dram_tensor("x_loc", [npcp, fin], BF16, kind="ExternalInput")
    packed_d = nc.dram_tensor("packed", [128, t_chunks], I32,
                              kind="ExternalInput")
    wgt_d = nc.dram_tensor("wgt", [128, t_chunks], BF16, kind="ExternalInput")
    w10 = nc.dram_tensor("w10", [fin, fhid], BF16, kind="ExternalInput")
    w11 = nc.dram_tensor("w11", [fin, fhid], BF16, kind="ExternalInput")
    wx0 = nc.dram_tensor("wx0", [fhid, fhid], BF16, kind="ExternalInput")
    wx1 = nc.dram_tensor("wx1", [fhid, fhid], BF16, kind="ExternalInput")
    w20 = nc.dram_tensor("w20", [fhid, fout], BF16, kind="ExternalInput")
    w21 = nc.dram_tensor("w21", [fhid, fout], BF16, kind="ExternalInput")
    b1_d = nc.dram_tensor("b1", [fhid, 1], F32, kind="ExternalInput")
    bx_d = nc.dram_tensor("bx", [fhid, 1], F32, kind="ExternalInput")
    b2_d = nc.dram_tensor("b2", [fout, 1], F32, kind="ExternalInput")
    iota_d = nc.dram_tensor("iota", [128, 128], BF16, kind="ExternalInput")
    id128_d = nc.dram_tensor("id128", [128, 128], BF16, kind="ExternalInput")
    id64_d = nc.dram_tensor("id64", [fhid, fhid], BF16, kind="ExternalInput")
    out_d = nc.dram_tensor("out", [fout, npcp], BF16, kind="ExternalOutput")

    xrows = nc.dram_tensor("xrows", [npcp, fin], BF16)
    rows1 = nc.dram_tensor("rows1", [npcp, fhid], BF16)
    rows2 = nc.dram_tensor("rows2", [npcp, fhid], BF16)
    tab1 = nc.dram_tensor("tab1", [np_all, fin], BF16, addr_space="Shared")
    tab2 = nc.dram_tensor("tab2", [np_all, fhid], BF16, addr_space="Shared")
    tab3 = nc.dram_tensor("tab3", [np_all, fhid], BF16, addr_space="Shared")

    rg = [list(range(M_CORES))]

    with ExitStack() as ctx:
        tc = ctx.enter_context(tile.TileContext(nc))
        const = ctx.enter_context(tc.tile_pool(name="const", bufs=1))
        xload = ctx.enter_context(tc.tile_pool(name="xload", bufs=4))
        featp = ctx.enter_context(tc.tile_pool(name="featp", bufs=10))
        meta = ctx.enter_context(tc.tile_pool(name="meta", bufs=3))
        ohp = ctx.enter_context(tc.tile_pool(name="ohp", bufs=4))
        txp = ctx.enter_context(tc.tile_pool(name="txp", bufs=3))
        otp = ctx.enter_context(tc.tile_pool(name="otp", bufs=3))
        stg = ctx.enter_context(tc.tile_pool(name="stg", bufs=3))
        psA = ctx.enter_context(tc.tile_pool(name="psA", bufs=3, space="PSUM"))
        psB = ctx.enter_context(tc.tile_pool(name="psB", bufs=2, space="PSUM"))
        psX = ctx.enter_context(tc.tile_pool(name="psX", bufs=1, space="PSUM"))
        psT = ctx.enter_context(tc.tile_pool(name="psT", bufs=2, space="PSUM"))

        def load_const(dram, shape, name, dtype=BF16):
            t = const.tile(shape, dtype, tag=name)
            nc.sync.dma_start(out=t[:], in_=dram[:, :])
            return t

        iota_t = load_const(iota_d, [128, 128], "iota")
        id128_t = load_const(id128_d, [128, 128], "id128")
        id64_t = load_const(id64_d, [fhid, fhid], "id64")
        w10_t = load_const(w10, [fin, fhid], "w10")
        w11_t = load_const(w11, [fin, fhid], "w11")
        wx0_t = load_const(wx0, [fhid, fhid], "wx0")
        wx1_t = load_const(wx1, [fhid, fhid], "wx1")
        w20_t = load_const(w20, [fhid, fout], "w20")
        w21_t = load_const(w21, [fhid, fout], "w21")
        b1_t = load_const(b1_d, [fhid, 1], "b1", F32)
        bx_t = load_const(bx_d, [fhid, 1], "bx", F32)
        b2_t = load_const(b2_d, [fout, 1], "b2", F32)

        # running feature maps, transposed layout, resident in SBUF
        xT_sb = const.tile([fin, npcp], BF16, tag="xT")
        hT1_sb = const.tile([fhid, npcp], BF16, tag="hT1")
        hT2_sb = const.tile([fhid, npcp], BF16, tag="hT2")

        # prologue: stage x rows to internal DRAM (AllGather source) and
        # build x^T tiles in SBUF for the layer-1 dense term
        for t in range(n_tiles):
            xt = xload.tile([128, fin], BF16, tag="xt")
            nc.sync.dma_start(out=xt[:], in_=x_loc_d[t * 128:(t + 1) * 128, :])
            nc.sync.dma_start(out=xrows[t * 128:(t + 1) * 128, :], in_=xt[:])
            px = psX.tile([fin, 128], BF16, tag="px")
            nc.tensor.transpose(out=px[:], in_=xt[:], identity=id128_t[:])
            nc.scalar.activation(
                xT_sb[:, t * 128:(t + 1) * 128], px[:],
                mybir.ActivationFunctionType.Copy,
            )
        nc.gpsimd.collective_compute(
            "AllGather",
            mybir.AluOpType.bypass,
            replica_groups=rg,
            ins=[xrows[:, :]],
            outs=[tab1[:, :]],
        )

        layers = [
            dict(table=tab1, rhs_sb=xT_sb, W0=w10_t, W1=w11_t, b=b1_t,
                 fo=fhid, hT_next=hT1_sb, rows=rows1, tab_next=tab2),
            dict(table=tab2, rhs_sb=hT1_sb, W0=wx0_t, W1=wx1_t, b=bx_t,
                 fo=fhid, hT_next=hT2_sb, rows=rows2, tab_next=tab3),
            dict(table=tab3, rhs_sb=hT2_sb, W0=w20_t, W1=w21_t, b=b2_t,
                 fo=fout, hT_next=None, rows=None, tab_next=None),
        ]

        for li, L in enumerate(layers):
            fo = L["fo"]
            offs_t = pos_b = w_t = None
            for j, (t, first, last) in enumerate(sched):
                if j % MG == 0:
                    mw = min(MG, t_chunks - j)
                    pk = meta.tile([128, MG], I32, tag="pk")
                    nc.sync.dma_start(out=pk[:, :mw],
                                      in_=packed_d[:, j:j + mw])
                    offs_t = meta.tile([128, MG], I32, tag="offs")
                    nc.vector.tensor_scalar(
                        out=offs_t[:, :mw], in0=pk[:, :mw],
                        scalar1=7, scalar2=None,
                        op0=mybir.AluOpType.logical_shift_right,
                    )
                    pos_i = meta.tile([128, MG], I32, tag="posi")
                    nc.vector.tensor_scalar(
                        out=pos_i[:, :mw], in0=pk[:, :mw],
                        scalar1=127, scalar2=None,
                        op0=mybir.AluOpType.bitwise_and,
                    )
                    pos_b = meta.tile([128, MG], F32, tag="posb")
                    nc.vector.tensor_copy(out=pos_b[:, :mw],
                                          in_=pos_i[:, :mw])
                    w_b = meta.tile([128, MG], BF16, tag="wb")
                    nc.sync.dma_start(out=w_b[:, :mw],
                                      in_=wgt_d[:, j:j + mw])
                    w_t = meta.tile([128, MG], F32, tag="wt")
                    nc.vector.tensor_copy(out=w_t[:, :mw],
                                          in_=w_b[:, :mw])
                jm = j % MG
                ft = featp.tile([128, fin], BF16, tag="ft")
                nc.gpsimd.indirect_dma_start(
                    out=ft[:],
                    out_offset=None,
                    in_=L["table"][:, :],
                    in_offset=bass.IndirectOffsetOnAxis(
                        ap=offs_t[:, jm:jm + 1], axis=0
                    ),
                )
                oh = ohp.tile([128, 128], BF16, tag="oh")
                nc.vector.tensor_scalar(
                    out=oh[:],
                    in0=iota_t[:],
                    scalar1=pos_b[:, jm:jm + 1],
                    scalar2=w_t[:, jm:jm + 1],
                    op0=mybir.AluOpType.is_equal,
                    op1=mybir.AluOpType.mult,
                )
                if first:
                    pa = psA.tile([fhid, 128], F32, tag="pa")
                nc.tensor.matmul(
                    pa[:], lhsT=ft[:], rhs=oh[:], start=first, stop=last
                )
                if last:
                    txT = txp.tile([fhid, 128], BF16, tag="tx")
                    nc.scalar.activation(
                        txT[:], pa[:], mybir.ActivationFunctionType.Copy
                    )
                    pb = psB.tile([fo, 128], F32, tag="pb")
                    nc.tensor.matmul(
                        pb[:], lhsT=L["W0"][:],
                        rhs=L["rhs_sb"][:, t * 128:(t + 1) * 128],
                        start=True, stop=False,
                    )
                    nc.tensor.matmul(pb[:], lhsT=L["W1"][:], rhs=txT[:],
                                     start=False, stop=True)
                    if L["hT_next"] is not None:
                        osl = L["hT_next"][:, t * 128:(t + 1) * 128]
                        nc.scalar.activation(
                            osl, pb[:],
                            mybir.ActivationFunctionType.Relu,
                            bias=L["b"][:],
                        )
                        pt = psT.tile([128, fhid], BF16, tag="pt")
                        nc.tensor.transpose(
                            out=pt[:], in_=osl, identity=id64_t[:]
                        )
                        st = stg.tile([128, fhid], BF16, tag="st")
                        nc.scalar.activation(
                            st[:], pt[:], mybir.ActivationFunctionType.Copy
                        )
                        nc.sync.dma_start(
                            out=L["rows"][t * 128:(t + 1) * 128, :],
                            in_=st[:],
                        )
                    else:
                        ot = otp.tile([fout, 128], BF16, tag="ot")
                        nc.scalar.activation(
                            ot[:], pb[:],
                            mybir.ActivationFunctionType.Identity,
                            bias=L["b"][:],
                        )
                        nc.sync.dma_start(
                            out=out_d[:, t * 128:(t + 1) * 128], in_=ot[:]
                        )
            if L["tab_next"] is not None:
                nc.gpsimd.collective_compute(
                    "AllGather",
                    mybir.AluOpType.bypass,
                    replica_groups=rg,
                    ins=[L["rows"][:, :]],
                    outs=[L["tab_next"][:, :]],
                )

    nc.compile()
    return nc


# ------------------------------------------------------------------ runner
def make_in_maps(inputs, n_nodes, npc, npcp, fin, fhid, fout, per_core):
    x = np.asarray(inputs["x"], dtype=np.float32)
    iota = np.broadcast_to(
        np.arange(128, dtype=np.float32), (128, 128)
    ).astype(NP_BF16)
    common = dict(
        w10=np.asarray(inputs["W1_0"], np.float32).astype(NP_BF16),
        w11=np.asarray(inputs["W1_1"], np.float32).astype(NP_BF16),
        wx0=np.asarray(inputs["Wx_0"], np.float32).astype(NP_BF16),
        wx1=np.asarray(inputs["Wx_1"], np.float32).astype(NP_BF16),
        w20=np.asarray(inputs["W2_0"], np.float32).astype(NP_BF16),
        w21=np.asarray(inputs["W2_1"], np.float32).astype(NP_BF16),
        b1=np.asarray(inputs["b1"], np.float32).reshape(fhid, 1),
        bx=np.asarray(inputs["bx"], np.float32).reshape(fhid, 1),
        b2=np.asarray(inputs["b2"], np.float32).reshape(fout, 1),
        iota=iota,
        id128=np.eye(128, dtype=np.float32).astype(NP_BF16),
        id64=np.eye(fhid, dtype=np.float32).astype(NP_BF16),
    )
    in_maps = []
    for c in range(M_CORES):
        x_loc = np.zeros((npcp, fin), dtype=NP_BF16)
        x_loc[:npc] = x[c * npc:(c + 1) * npc].astype(NP_BF16)
        in_maps.append(dict(common, x_loc=x_loc, **per_core[c]))
    return in_maps


def run(inputs, n_nodes, fin, fhid, fout, trace=False, trace_kwargs=None,
        timeit=0):
    npc = n_nodes // M_CORES
    npcp = int(math.ceil(npc / 128.0)) * 128

    adj = np.asarray(inputs["adj"], dtype=np.int32)
    sched, per_core = host_prep(adj, n_nodes, npc, npcp)
    nc = build_program(sched, npcp, fin, fhid, fout)
    in_maps = make_in_maps(
        inputs, n_nodes, npc, npcp, fin, fhid, fout, per_core
    )
    res = run_bass_kernel_spmd(
        nc,
        in_maps,
        core_ids=list(range(M_CORES)),
        trace=trace,
        **(trace_kwargs or {}),
    )
    times = []
    for _ in range(timeit):
        t0 = time.perf_counter()
        run_bass_kernel_spmd(nc, in_maps, core_ids=list(range(M_CORES)))
        times.append(time.perf_counter() - t0)
    if times:
        print("repeat wall times (s):", [f"{t:.3f}" for t in times])
        global LAST_TIMES
        LAST_TIMES = times
    out = np.concatenate(
        [
            np.asarray(res.results[c]["out"])[:, :npc].T.astype(np.float32)
            for c in range(M_CORES)
        ],
        axis=0,
    )
    return out, res


def kernel(**inputs):
    out, _ = run(inputs, n_nodes=100000, fin=64, fhid=64, fout=16)
    return out


# revision 8
# speedup vs baseline: 7.6849x; 1.0705x over previous
"""ChebGCN (K=2, 3 layers) Trainium2 kernel — 8-core SPMD.

Sharding: nodes are split across 8 cores (12500/core, padded to 12544 for
128 alignment). Edges are bucketed by destination core, sorted by
destination node and packed into 128-edge chunks aligned to 128-node
destination tiles; chunk counts per tile are equalized across cores so all
8 cores run one SPMD program.

Host->device traffic dominates wall time (the PJRT tunnel moves ~40 MB/s
and charges ~10 ms per sharded array), so each core receives exactly ONE
int32 tensor that packs, bit-cast per region: per-edge-slot metadata
(source_index*128 + dest_pos as int32, weight as bf16), the core's bf16 x
slice laid out in 128-row tiles, and all weights/biases/iota/identity
constants. The full gather table is built on device by an 8-core AllGather
of the x slices; layer-1 dense-term inputs (x^T tiles) are produced on
device by PE transposes. Per propagate, each core gathers the 128 source
rows of a chunk with one indirect DMA, builds a weighted one-hot on DVE
((iota == dest_pos) * w) and the TensorEngine accumulates feat^T @ onehot
into PSUM, producing segment sums in transposed layout. Dense 64-wide
weight matmuls + bias/relu stay in transposed layout with the running
feature map resident in SBUF; per 128-node tile the result is
PE-transposed back to row-major, DMA'd to DRAM and AllGathered to rebuild
the gather table for the next layer. Output leaves the device as bf16.
"""

import sys

for _p in ("/opt/trn_rl_repo",):
    if _p not in sys.path:
        sys.path.insert(0, _p)

import math
import time
from contextlib import ExitStack

import ml_dtypes
import numpy as np

import concourse.bacc as bacc
import concourse.bass as bass
import concourse.mybir as mybir
import concourse.tile as tile
from concourse.bass_utils import run_bass_kernel_spmd

F32 = mybir.dt.float32
I32 = mybir.dt.int32
BF16 = mybir.dt.bfloat16
NP_BF16 = ml_dtypes.bfloat16

M_CORES = 8
MG = 64  # chunks per metadata (packed/wgt) load
XW = 33  # int32 cols per x tile in the blob (66 bf16: 64 feat + dis + pad)
CW616 = 616  # bf16 const-region cols
# bf16 col offsets inside the const region
IOTA_C, ID128_C, W10_C, W11_C, WX0_C, WX1_C = 0, 128, 256, 320, 384, 448
W20_C, W21_C, ID64_C, B1_C, BX_C, B2_C = 512, 528, 544, 608, 609, 610
LAST_TIMES = []  # wall times of repeat runs (filled by run(timeit=N))


def blob_geom(t_chunks, n_tiles):
    """int32-column offsets of the blob regions."""
    t2 = t_chunks + (t_chunks & 1)
    xb = t_chunks            # x-tile region start
    cb = xb + n_tiles * XW   # const region start
    wb = cb + CW616 // 2     # edge-weight region start
    w32 = wb + t2 // 2
    return xb, cb, wb, w32


# ---------------------------------------------------------------- host prep
def host_prep(adj, n_nodes, npc, npcp):
    """Bucket/sort/pad edges -> per-core slot arrays + shared chunk schedule.

    Returns (sched, per_core, dis): sched[j] = (tile_idx, is_first, is_last)
    per 128-edge chunk (identical across cores); per_core[c] has
    packed = src_padded_index*128 + dest_pos (int32) and wgt (f32),
    each [128, T]; dis = per-node deg^-1/2.
    """
    n_tiles = npcp // 128
    row = adj[0].astype(np.int64)
    col = adj[1].astype(np.int64)

    deg = np.bincount(row, minlength=n_nodes).astype(np.float64)
    dis = np.where(deg > 0, 1.0 / np.sqrt(np.maximum(deg, 1)), 0.0).astype(
        np.float32
    )
    w_all = (-(dis[row] * dis[col])).astype(np.float32)
    colp = (col // npc) * npcp + (col % npc)

    core_of = row // npc
    per_core_raw = []
    counts = np.zeros((M_CORES, n_tiles), dtype=np.int64)
    for c in range(M_CORES):
        sel = np.nonzero(core_of == c)[0]
        r_loc = row[sel] - c * npc
        order = np.argsort(r_loc, kind="stable")
        sel = sel[order]
        per_core_raw.append((r_loc[order], colp[sel], w_all[sel]))
        counts[c] = np.bincount(r_loc[order] // 128, minlength=n_tiles)

    nch = np.maximum(np.ceil(counts / 128.0).astype(np.int64).max(axis=0), 1)
    t_chunks = int(nch.sum())

    sched = []
    for t in range(n_tiles):
        for k in range(int(nch[t])):
            sched.append((t, k == 0, k == int(nch[t]) - 1))
    tile_base = np.concatenate([[0], np.cumsum(nch)[:-1]]) * 128

    per_core = []
    for c in range(M_CORES):
        r_loc, cp, wc = per_core_raw[c]
        packed = np.zeros(t_chunks * 128, dtype=np.int32)
        wgt = np.zeros(t_chunks * 128, dtype=np.float32)
        t_of = r_loc // 128
        cnt = np.bincount(t_of, minlength=n_tiles)
        idx_within = np.zeros_like(r_loc)
        start = 0
        for t in range(n_tiles):
            e = start + int(cnt[t])
            idx_within[start:e] = np.arange(e - start)
            start = e
        slots = tile_base[t_of] + idx_within
        packed[slots] = (cp * 128 + (r_loc - t_of * 128)).astype(np.int32)
        wgt[slots] = wc
        per_core.append(
            dict(
                packed=np.ascontiguousarray(
                    packed.reshape(t_chunks, 128).T
                ),
                wgt=np.ascontiguousarray(wgt.reshape(t_chunks, 128).T),
            )
        )
    return sched, per_core, dis


# ------------------------------------------------------------- bass program
def build_program(sched, npcp, fin, fhid, fout):
    n_tiles = npcp // 128
    np_all = npcp * M_CORES
    t_chunks = len(sched)
    xb, cbase, wb, w32 = blob_geom(t_chunks, n_tiles)

    nc = bacc.Bacc(
        "TRN2",
        target_bir_lowering=False,
        debug=False,
        enable_asserts=True,
        num_devices=M_CORES,
    )

    blob_d = nc.dram_tensor("blob", [128, w32], I32, kind="ExternalInput")
    out_d = nc.dram_tensor("out", [fout, npcp], BF16, kind="ExternalOutput")

    xrows = nc.dram_tensor("xrows", [npcp, fin], BF16)
    rows1 = nc.dram_tensor("rows1", [npcp, fhid], BF16)
    rows2 = nc.dram_tensor("rows2", [npcp, fhid], BF16)
    tab1 = nc.dram_tensor("tab1", [np_all, fin], BF16, addr_space="Shared")
    tab2 = nc.dram_tensor("tab2", [np_all, fhid], BF16, addr_space="Shared")
    tab3 = nc.dram_tensor("tab3", [np_all, fhid], BF16, addr_space="Shared")

    rg = [list(range(M_CORES))]

    with ExitStack() as ctx:
        tc = ctx.enter_context(tile.TileContext(nc))
        const = ctx.enter_context(tc.tile_pool(name="const", bufs=1))
        xload = ctx.enter_context(tc.tile_pool(name="xload", bufs=4))
        featp = ctx.enter_context(tc.tile_pool(name="featp", bufs=10))
        meta = ctx.enter_context(tc.tile_pool(name="meta", bufs=3))
        ohp = ctx.enter_context(tc.tile_pool(name="ohp", bufs=4))
        txp = ctx.enter_context(tc.tile_pool(name="txp", bufs=3))
        otp = ctx.enter_context(tc.tile_pool(name="otp", bufs=3))
        stg = ctx.enter_context(tc.tile_pool(name="stg", bufs=3))
        psA = ctx.enter_context(tc.tile_pool(name="psA", bufs=3, space="PSUM"))
        psB = ctx.enter_context(tc.tile_pool(name="psB", bufs=2, space="PSUM"))
        psX = ctx.enter_context(tc.tile_pool(name="psX", bufs=1, space="PSUM"))
        psT = ctx.enter_context(tc.tile_pool(name="psT", bufs=2, space="PSUM"))

        # const region: one DMA, then slice views
        cb = const.tile([128, CW616], BF16, tag="cb")
        nc.sync.dma_start(
            out=cb[:],
            in_=blob_d[:, cbase:cbase + CW616 // 2].bitcast(BF16),
        )
        iota_t = cb[:, IOTA_C:IOTA_C + 128]
        id128_t = cb[:, ID128_C:ID128_C + 128]
        id64_t = cb[0:fhid, ID64_C:ID64_C + fhid]
        w10_t = cb[0:fin, W10_C:W10_C + fhid]
        w11_t = cb[0:fin, W11_C:W11_C + fhid]
        wx0_t = cb[0:fhid, WX0_C:WX0_C + fhid]
        wx1_t = cb[0:fhid, WX1_C:WX1_C + fhid]
        w20_t = cb[0:fhid, W20_C:W20_C + fout]
        w21_t = cb[0:fhid, W21_C:W21_C + fout]
        b1_t = const.tile([fhid, 1], F32, tag="b1")
        nc.vector.tensor_copy(out=b1_t[:], in_=cb[0:fhid, B1_C:B1_C + 1])
        bx_t = const.tile([fhid, 1], F32, tag="bx")
        nc.vector.tensor_copy(out=bx_t[:], in_=cb[0:fhid, BX_C:BX_C + 1])
        b2_t = const.tile([fout, 1], F32, tag="b2")
        nc.vector.tensor_copy(out=b2_t[:], in_=cb[0:fout, B2_C:B2_C + 1])

        # running feature maps, transposed layout, resident in SBUF
        xT_sb = const.tile([fin, npcp], BF16, tag="xT")
        hT1_sb = const.tile([fhid, npcp], BF16, tag="hT1")
        hT2_sb = const.tile([fhid, npcp], BF16, tag="hT2")

        # prologue: stage x rows to internal DRAM (AllGather source) and
        # build x^T tiles in SBUF for the layer-1 dense term
        for t in range(n_tiles):
            xt = xload.tile([128, 2 * XW], BF16, tag="xt")
            nc.sync.dma_start(
                out=xt[:],
                in_=blob_d[:, xb + XW * t:xb + XW * (t + 1)].bitcast(BF16),
            )
            nc.sync.dma_start(
                out=xrows[t * 128:(t + 1) * 128, :], in_=xt[:, 0:fin]
            )
            px = psX.tile([fin, 128], BF16, tag="px")
            nc.tensor.transpose(
                out=px[:], in_=xt[:, 0:fin], identity=id128_t
            )
            nc.scalar.activation(
                xT_sb[:, t * 128:(t + 1) * 128], px[:],
                mybir.ActivationFunctionType.Copy,
            )
        nc.gpsimd.collective_compute(
            "AllGather",
            mybir.AluOpType.bypass,
            replica_groups=rg,
            ins=[xrows[:, :]],
            outs=[tab1[:, :]],
        )

        layers = [
            dict(table=tab1, rhs_sb=xT_sb, W0=w10_t, W1=w11_t, b=b1_t,
                 fo=fhid, hT_next=hT1_sb, rows=rows1, tab_next=tab2),
            dict(table=tab2, rhs_sb=hT1_sb, W0=wx0_t, W1=wx1_t, b=bx_t,
                 fo=fhid, hT_next=hT2_sb, rows=rows2, tab_next=tab3),
            dict(table=tab3, rhs_sb=hT2_sb, W0=w20_t, W1=w21_t, b=b2_t,
                 fo=fout, hT_next=None, rows=None, tab_next=None),
        ]

        for li, L in enumerate(layers):
            fo = L["fo"]
            offs_t = pos_b = w_t = None
            for j, (t, first, last) in enumerate(sched):
                if j % MG == 0:
                    mw = min(MG, t_chunks - j)
                    mw2 = mw + (mw & 1)
                    pk = meta.tile([128, MG], I32, tag="pk")
                    nc.sync.dma_start(out=pk[:, :mw],
                                      in_=blob_d[:, j:j + mw])
                    offs_t = meta.tile([128, MG], I32, tag="offs")
                    nc.vector.tensor_scalar(
                        out=offs_t[:, :mw], in0=pk[:, :mw],
                        scalar1=7, scalar2=None,
                        op0=mybir.AluOpType.logical_shift_right,
                    )
                    pos_i = meta.tile([128, MG], I32, tag="posi")
                    nc.vector.tensor_scalar(
                        out=pos_i[:, :mw], in0=pk[:, :mw],
                        scalar1=127, scalar2=None,
                        op0=mybir.AluOpType.bitwise_and,
                    )
                    pos_b = meta.tile([128, MG], F32, tag="posb")
                    nc.vector.tensor_copy(out=pos_b[:, :mw],
                                          in_=pos_i[:, :mw])
                    w_b = meta.tile([128, MG], BF16, tag="wb")
                    nc.sync.dma_start(
                        out=w_b[:, :mw2],
                        in_=blob_d[
                            :, wb + j // 2:wb + (j + mw2) // 2
                        ].bitcast(BF16),
                    )
                    w_t = meta.tile([128, MG], F32, tag="wt")
                    nc.vector.tensor_copy(out=w_t[:, :mw],
                                          in_=w_b[:, :mw])
                jm = j % MG
                ft = featp.tile([128, fin], BF16, tag="ft")
                nc.gpsimd.indirect_dma_start(
                    out=ft[:],
                    out_offset=None,
                    in_=L["table"][:, :],
                    in_offset=bass.IndirectOffsetOnAxis(
                        ap=offs_t[:, jm:jm + 1], axis=0
                    ),
                )
                oh = ohp.tile([128, 128], BF16, tag="oh")
                nc.vector.tensor_scalar(
                    out=oh[:],
                    in0=iota_t,
                    scalar1=pos_b[:, jm:jm + 1],
                    scalar2=w_t[:, jm:jm + 1],
                    op0=mybir.AluOpType.is_equal,
                    op1=mybir.AluOpType.mult,
                )
                if first:
                    pa = psA.tile([fhid, 128], F32, tag="pa")
                nc.tensor.matmul(
                    pa[:], lhsT=ft[:], rhs=oh[:], start=first, stop=last
                )
                if last:
                    txT = txp.tile([fhid, 128], BF16, tag="tx")
                    nc.scalar.activation(
                        txT[:], pa[:], mybir.ActivationFunctionType.Copy
                    )
                    pb = psB.tile([fo, 128], F32, tag="pb")
                    nc.tensor.matmul(
                        pb[:], lhsT=L["W0"],
                        rhs=L["rhs_sb"][:, t * 128:(t + 1) * 128],
                        start=True, stop=False,
                    )
                    nc.tensor.matmul(pb[:], lhsT=L["W1"], rhs=txT[:],
                                     start=False, stop=True)
                    if L["hT_next"] is not None:
                        osl = L["hT_next"][:, t * 128:(t + 1) * 128]
                        nc.scalar.activation(
                            osl, pb[:],
                            mybir.ActivationFunctionType.Relu,
                            bias=L["b"][:],
                        )
                        pt = psT.tile([128, fhid], BF16, tag="pt")
                        nc.tensor.transpose(
                            out=pt[:], in_=osl, identity=id64_t
                        )
                        st = stg.tile([128, fhid], BF16, tag="st")
                        nc.scalar.activation(
                            st[:], pt[:], mybir.ActivationFunctionType.Copy
                        )
                        nc.sync.dma_start(
                            out=L["rows"][t * 128:(t + 1) * 128, :],
                            in_=st[:],
                        )
                    else:
                        ot = otp.tile([fout, 128], BF16, tag="ot")
                        nc.scalar.activation(
                            ot[:], pb[:],
                            mybir.ActivationFunctionType.Identity,
                            bias=L["b"][:],
                        )
                        nc.sync.dma_start(
                            out=out_d[:, t * 128:(t + 1) * 128], in_=ot[:]
                        )
            if L["tab_next"] is not None:
                nc.gpsimd.collective_compute(
                    "AllGather",
                    mybir.AluOpType.bypass,
                    replica_groups=rg,
                    ins=[L["rows"][:, :]],
                    outs=[L["tab_next"][:, :]],
                )

    nc.compile()
    return nc


# ------------------------------------------------------------------ runner
def make_in_maps(inputs, n_nodes, npc, npcp, fin, fhid, fout, per_core, dis):
    n_tiles = npcp // 128
    t_chunks = per_core[0]["packed"].shape[1]
    t2 = t_chunks + (t_chunks & 1)
    x = np.asarray(inputs["x"], dtype=np.float32)

    consts = np.zeros((128, CW616), dtype=NP_BF16)
    consts[:, IOTA_C:IOTA_C + 128] = np.arange(128, dtype=np.float32)[
        None, :
    ].astype(NP_BF16)
    consts[:, ID128_C:ID128_C + 128] = np.eye(128, dtype=np.float32).astype(
        NP_BF16
    )
    consts[0:fhid, ID64_C:ID64_C + fhid] = np.eye(
        fhid, dtype=np.float32
    ).astype(NP_BF16)
    for name, c0 in (("W1_0", W10_C), ("W1_1", W11_C), ("Wx_0", WX0_C),
                     ("Wx_1", WX1_C), ("W2_0", W20_C), ("W2_1", W21_C)):
        w = np.asarray(inputs[name], np.float32).astype(NP_BF16)
        consts[0:w.shape[0], c0:c0 + w.shape[1]] = w
    consts[0:fhid, B1_C] = np.asarray(inputs["b1"], np.float32).astype(
        NP_BF16
    )
    consts[0:fhid, BX_C] = np.asarray(inputs["bx"], np.float32).astype(
        NP_BF16
    )
    consts[0:fout, B2_C] = np.asarray(inputs["b2"], np.float32).astype(
        NP_BF16
    )

    in_maps = []
    for c in range(M_CORES):
        xp = np.zeros((npcp, 66), dtype=np.float32)
        xp[:npc, :fin] = x[c * npc:(c + 1) * npc]
        xp[:npc, fin] = dis[c * npc:(c + 1) * npc]
        xtiles = np.ascontiguousarray(
            xp.reshape(n_tiles, 128, 66).transpose(1, 0, 2).reshape(
                128, n_tiles * 66
            )
        ).astype(NP_BF16)
        wpad = np.zeros((128, t2), dtype=NP_BF16)
        wpad[:, :t_chunks] = per_core[c]["wgt"].astype(NP_BF16)
        payload = np.concatenate([xtiles, consts, wpad], axis=1)
        blob = np.concatenate(
            [per_core[c]["packed"], payload.view(np.int32)], axis=1
        )
        in_maps.append(dict(blob=np.ascontiguousarray(blob)))
    return in_maps


def run(inputs, n_nodes, fin, fhid, fout, trace=False, trace_kwargs=None,
        timeit=0):
    npc = n_nodes // M_CORES
    npcp = int(math.ceil(npc / 128.0)) * 128

    adj = np.asarray(inputs["adj"], dtype=np.int32)
    sched, per_core, dis = host_prep(adj, n_nodes, npc, npcp)
    nc = build_program(sched, npcp, fin, fhid, fout)
    in_maps = make_in_maps(
        inputs, n_nodes, npc, npcp, fin, fhid, fout, per_core, dis
    )
    res = run_bass_kernel_spmd(
        nc,
        in_maps,
        core_ids=list(range(M_CORES)),
        trace=trace,
        **(trace_kwargs or {}),
    )
    times = []
    for _ in range(timeit):
        t0 = time.perf_counter()
        run_bass_kernel_spmd(nc, in_maps, core_ids=list(range(M_CORES)))
        times.append(time.perf_counter() - t0)
    if times:
        print("repeat wall times (s):", [f"{t:.3f}" for t in times])
        global LAST_TIMES
        LAST_TIMES = times
    out = np.concatenate(
        [
            np.asarray(res.results[c]["out"])[:, :npc].T.astype(np.float32)
            for c in range(M_CORES)
        ],
        axis=0,
    )
    return out, res


def kernel(**inputs):
    out, _ = run(inputs, n_nodes=100000, fin=64, fhid=64, fout=16)
    return out


# revision 9
# speedup vs baseline: 11.8987x; 1.5483x over previous
"""ChebGCN (K=2, 3 layers) Trainium2 kernel — 8-core SPMD.

Sharding: nodes are split across 8 cores (12500/core, padded to 12544 for
128 alignment). Edges are bucketed by destination core, sorted by
destination node and packed into 128-edge chunks aligned to 128-node
destination tiles; chunk counts per tile are equalized across cores so all
8 cores run one SPMD program.

Host->device traffic and per-instruction dispatch dominate wall time, so
(a) each core receives exactly ONE int32 tensor packing, bit-cast per
region: per-edge-slot metadata (source_index*128 + dest_pos int32, weight
bf16), the core's bf16 x slice, and all weight/bias/iota constants; and
(b) the program minimizes instruction count: metadata is unpacked once
into persistent SBUF (not per layer), one-hot scatter matrices are built
16 chunks per DVE op via broadcast tensor_tensor, gathered features are
weighted 16 chunks per op, dense 64-wide weight matmuls + bias/relu are
batched 4 node-tiles (512 cols) per op, row-major tables are produced by
transposed-access-pattern DMA stores, and x^T arrives via one whole-table
DMA transpose. The full gather table is rebuilt per layer by an 8-core
AllGather; per 128-edge chunk one indirect DMA gathers the 128 source
rows and one PE matmul accumulates feat^T @ onehot into PSUM, yielding
segment sums in transposed layout. Output leaves the device as bf16.
"""

import sys

for _p in ("/opt/trn_rl_repo",):
    if _p not in sys.path:
        sys.path.insert(0, _p)

import math
import time
from contextlib import ExitStack

import ml_dtypes
import numpy as np

import concourse.bacc as bacc
import concourse.bass as bass
import concourse.mybir as mybir
import concourse.tile as tile
from concourse.bass_utils import run_bass_kernel_spmd

F32 = mybir.dt.float32
I32 = mybir.dt.int32
BF16 = mybir.dt.bfloat16
NP_BF16 = ml_dtypes.bfloat16

M_CORES = 8
G = 16        # chunks per one-hot/weight DVE batch
TG = 4        # node tiles per dense matmul batch (512 cols)
XW = 32       # int32 cols per x tile in the blob (64 bf16 feats)
CW = 420      # bf16 const-region cols (even)
IOTA_C, W10_C, W11_C, WX0_C, WX1_C = 0, 128, 192, 256, 320
W20_C, W21_C, B1_C, BX_C, B2_C = 384, 400, 416, 417, 418
LAST_TIMES = []  # wall times of repeat runs (filled by run(timeit=N))


def blob_geom(t_chunks, n_tiles):
    """int32-column offsets of the blob regions."""
    t2 = t_chunks + (t_chunks & 1)
    xb = t_chunks            # x-tile region start
    cb = xb + n_tiles * XW   # const region start
    wb = cb + CW // 2        # edge-weight region start
    w32 = wb + t2 // 2
    return xb, cb, wb, w32


# ---------------------------------------------------------------- host prep
def host_prep(adj, n_nodes, npc, npcp):
    """Bucket/sort/pad edges -> per-core slot arrays + shared chunk schedule.

    Returns (sched, per_core): sched[j] = (tile_idx, is_first, is_last)
    per 128-edge chunk (identical across cores); per_core[c] has
    packed = src_padded_index*128 + dest_pos (int32) and wgt (f32),
    each [128, T].
    """
    n_tiles = npcp // 128
    row = adj[0].astype(np.int64)
    col = adj[1].astype(np.int64)

    deg = np.bincount(row, minlength=n_nodes).astype(np.float64)
    dis = np.where(deg > 0, 1.0 / np.sqrt(np.maximum(deg, 1)), 0.0).astype(
        np.float32
    )
    w_all = (-(dis[row] * dis[col])).astype(np.float32)
    colp = (col // npc) * npcp + (col % npc)

    core_of = row // npc
    per_core_raw = []
    counts = np.zeros((M_CORES, n_tiles), dtype=np.int64)
    for c in range(M_CORES):
        sel = np.nonzero(core_of == c)[0]
        r_loc = row[sel] - c * npc
        order = np.argsort(r_loc, kind="stable")
        sel = sel[order]
        per_core_raw.append((r_loc[order], colp[sel], w_all[sel]))
        counts[c] = np.bincount(r_loc[order] // 128, minlength=n_tiles)

    nch = np.maximum(np.ceil(counts / 128.0).astype(np.int64).max(axis=0), 1)
    t_chunks = int(nch.sum())

    sched = []
    for t in range(n_tiles):
        for k in range(int(nch[t])):
            sched.append((t, k == 0, k == int(nch[t]) - 1))
    tile_base = np.concatenate([[0], np.cumsum(nch)[:-1]]) * 128

    per_core = []
    for c in range(M_CORES):
        r_loc, cp, wc = per_core_raw[c]
        packed = np.zeros(t_chunks * 128, dtype=np.int32)
        wgt = np.zeros(t_chunks * 128, dtype=np.float32)
        t_of = r_loc // 128
        cnt = np.bincount(t_of, minlength=n_tiles)
        idx_within = np.zeros_like(r_loc)
        start = 0
        for t in range(n_tiles):
            e = start + int(cnt[t])
            idx_within[start:e] = np.arange(e - start)
            start = e
        slots = tile_base[t_of] + idx_within
        packed[slots] = (cp * 128 + (r_loc - t_of * 128)).astype(np.int32)
        wgt[slots] = wc
        per_core.append(
            dict(
                packed=np.ascontiguousarray(
                    packed.reshape(t_chunks, 128).T
                ),
                wgt=np.ascontiguousarray(wgt.reshape(t_chunks, 128).T),
            )
        )
    return sched, per_core


# ------------------------------------------------------------- bass program
def build_program(sched, npcp, fin, fhid, fout):
    n_tiles = npcp // 128
    np_all = npcp * M_CORES
    t_chunks = len(sched)
    xb, cbase, wbase, w32 = blob_geom(t_chunks, n_tiles)
    t2 = t_chunks + (t_chunks & 1)

    nc = bacc.Bacc(
        "TRN2",
        target_bir_lowering=False,
        debug=False,
        enable_asserts=False,
        num_devices=M_CORES,
    )

    blob_d = nc.dram_tensor("blob", [128, w32], I32, kind="ExternalInput")
    out_d = nc.dram_tensor("out", [fout, npcp], BF16, kind="ExternalOutput")

    xrows = nc.dram_tensor("xrows", [npcp, fin], BF16)
    rows1 = nc.dram_tensor("rows1", [npcp, fhid], BF16)
    rows2 = nc.dram_tensor("rows2", [npcp, fhid], BF16)
    tab1 = nc.dram_tensor("tab1", [np_all, fin], BF16, addr_space="Shared")
    tab2 = nc.dram_tensor("tab2", [np_all, fhid], BF16, addr_space="Shared")
    tab3 = nc.dram_tensor("tab3", [np_all, fhid], BF16, addr_space="Shared")

    rg = [list(range(M_CORES))]

    with ExitStack() as ctx:
        tc = ctx.enter_context(tile.TileContext(nc))
        const = ctx.enter_context(tc.tile_pool(name="const", bufs=1))
        meta = ctx.enter_context(tc.tile_pool(name="meta", bufs=2))
        featp = ctx.enter_context(tc.tile_pool(name="featp", bufs=4))
        ohp = ctx.enter_context(tc.tile_pool(name="ohp", bufs=3))
        ftwp = ctx.enter_context(tc.tile_pool(name="ftwp", bufs=3))
        txp = ctx.enter_context(tc.tile_pool(name="txp", bufs=3))
        otp = ctx.enter_context(tc.tile_pool(name="otp", bufs=2))
        psA = ctx.enter_context(tc.tile_pool(name="psA", bufs=3, space="PSUM"))
        psB = ctx.enter_context(tc.tile_pool(name="psB", bufs=2, space="PSUM"))

        # const region: one DMA, then slice views
        cb = const.tile([128, CW], BF16, tag="cb")
        nc.sync.dma_start(
            out=cb[:],
            in_=blob_d[:, cbase:cbase + CW // 2].bitcast(BF16),
        )
        iota_t = cb[:, IOTA_C:IOTA_C + 128]
        w10_t = cb[0:fin, W10_C:W10_C + fhid]
        w11_t = cb[0:fin, W11_C:W11_C + fhid]
        wx0_t = cb[0:fhid, WX0_C:WX0_C + fhid]
        wx1_t = cb[0:fhid, WX1_C:WX1_C + fhid]
        w20_t = cb[0:fhid, W20_C:W20_C + fout]
        w21_t = cb[0:fhid, W21_C:W21_C + fout]
        b1_t = const.tile([fhid, 1], F32, tag="b1")
        nc.vector.tensor_copy(out=b1_t[:], in_=cb[0:fhid, B1_C:B1_C + 1])
        bx_t = const.tile([fhid, 1], F32, tag="bx")
        nc.vector.tensor_copy(out=bx_t[:], in_=cb[0:fhid, BX_C:BX_C + 1])
        b2_t = const.tile([fout, 1], F32, tag="b2")
        nc.vector.tensor_copy(out=b2_t[:], in_=cb[0:fout, B2_C:B2_C + 1])

        # unpack all edge metadata once into persistent SBUF
        offs_all = const.tile([128, t_chunks], I32, tag="offs")
        pos_all = const.tile([128, t_chunks], BF16, tag="pos")
        w_all = const.tile([128, t2], BF16, tag="wal")
        nc.sync.dma_start(
            out=w_all[:],
            in_=blob_d[:, wbase:wbase + t2 // 2].bitcast(BF16),
        )
        MU = 512
        for m0 in range(0, t_chunks, MU):
            mw = min(MU, t_chunks - m0)
            pk = meta.tile([128, MU], I32, tag="pk")
            nc.sync.dma_start(out=pk[:, :mw], in_=blob_d[:, m0:m0 + mw])
            nc.vector.tensor_scalar(
                out=offs_all[:, m0:m0 + mw], in0=pk[:, :mw],
                scalar1=7, scalar2=None,
                op0=mybir.AluOpType.logical_shift_right,
            )
            pos_i = meta.tile([128, MU], I32, tag="posi")
            nc.vector.tensor_scalar(
                out=pos_i[:, :mw], in0=pk[:, :mw],
                scalar1=127, scalar2=None,
                op0=mybir.AluOpType.bitwise_and,
            )
            nc.vector.tensor_copy(out=pos_all[:, m0:m0 + mw],
                                  in_=pos_i[:, :mw])

        # x prologue: bulk load, row-major store, transpose for dense term
        xa = const.tile([128, n_tiles * XW], I32, tag="xa")
        nc.sync.dma_start(out=xa[:],
                          in_=blob_d[:, xb:xb + n_tiles * XW])
        xa16 = xa[:].bitcast(BF16).rearrange("p (t f) -> p t f", f=2 * XW)
        nc.sync.dma_start(
            out=xrows[:, :].rearrange("(t p) f -> p t f", p=128),
            in_=xa16,
        )
        nc.gpsimd.collective_compute(
            "AllGather",
            mybir.AluOpType.bypass,
            replica_groups=rg,
            ins=[xrows[:, :]],
            outs=[tab1[:, :]],
        )
        xT_sb = const.tile([fin, npcp], BF16, tag="xT")
        nc.sync.dma_start_transpose(xT_sb[:], xrows[:, :])

        hT1_sb = const.tile([fhid, npcp], BF16, tag="hT1")
        hT2_sb = const.tile([fhid, npcp], BF16, tag="hT2")

        layers = [
            dict(table=tab1, rhs_sb=xT_sb, W0=w10_t, W1=w11_t, b=b1_t,
                 fo=fhid, hT_next=hT1_sb, rows=rows1, tab_next=tab2),
            dict(table=tab2, rhs_sb=hT1_sb, W0=wx0_t, W1=wx1_t, b=bx_t,
                 fo=fhid, hT_next=hT2_sb, rows=rows2, tab_next=tab3),
            dict(table=tab3, rhs_sb=hT2_sb, W0=w20_t, W1=w21_t, b=b2_t,
                 fo=fout, hT_next=None, rows=None, tab_next=None),
        ]

        for li, L in enumerate(layers):
            fo = L["fo"]
            pa = None
            for j0 in range(0, t_chunks, G):
                gw = min(G, t_chunks - j0)
                # gather 16 chunks into one SBUF tile
                ftg = featp.tile([128, G, fin], BF16, tag="ftg")
                for g in range(gw):
                    nc.gpsimd.indirect_dma_start(
                        out=ftg[:, g, :],
                        out_offset=None,
                        in_=L["table"][:, :],
                        in_offset=bass.IndirectOffsetOnAxis(
                            ap=offs_all[:, j0 + g:j0 + g + 1], axis=0
                        ),
                    )
                # one-hots for 16 chunks in one DVE op
                ohg = ohp.tile([128, G, 128], BF16, tag="ohg")
                nc.vector.tensor_tensor(
                    out=ohg[:, :gw, :],
                    in0=iota_t.unsqueeze(1).broadcast_to([128, gw, 128]),
                    in1=pos_all[:, j0:j0 + gw].unsqueeze(2).broadcast_to(
                        [128, gw, 128]
                    ),
                    op=mybir.AluOpType.is_equal,
                )
                # edge weights folded into gathered features, one DVE op
                ftw = ftwp.tile([128, G, fin], BF16, tag="ftw")
                nc.vector.tensor_tensor(
                    out=ftw[:, :gw, :],
                    in0=ftg[:, :gw, :],
                    in1=w_all[:, j0:j0 + gw].unsqueeze(2).broadcast_to(
                        [128, gw, fin]
                    ),
                    op=mybir.AluOpType.mult,
                )
                for g in range(gw):
                    t, first, last = sched[j0 + g]
                    tq = t % TG
                    if first and tq == 0:
                        pa = psA.tile([fhid, TG * 128], F32, tag="pa")
                    nc.tensor.matmul(
                        pa[:, tq * 128:(tq + 1) * 128],
                        lhsT=ftw[:, g, :], rhs=ohg[:, g, :],
                        start=first, stop=last,
                    )
                    if last and (tq == TG - 1 or t == n_tiles - 1):
                        q = t // TG
                        gw4 = (tq + 1) * 128
                        c0 = q * TG * 128
                        txg = txp.tile([fhid, TG * 128], BF16, tag="txg")
                        nc.scalar.activation(
                            txg[:, :gw4], pa[:, :gw4],
                            mybir.ActivationFunctionType.Copy,
                        )
                        pb = psB.tile([fo, TG * 128], F32, tag="pb")
                        nc.tensor.matmul(
                            pb[:, :gw4], lhsT=L["W0"],
                            rhs=L["rhs_sb"][:, c0:c0 + gw4],
                            start=True, stop=False,
                        )
                        nc.tensor.matmul(
                            pb[:, :gw4], lhsT=L["W1"], rhs=txg[:, :gw4],
                            start=False, stop=True,
                        )
                        if L["hT_next"] is not None:
                            osl = L["hT_next"][:, c0:c0 + gw4]
                            nc.scalar.activation(
                                osl, pb[:, :gw4],
                                mybir.ActivationFunctionType.Relu,
                                bias=L["b"][:],
                            )
                            nc.sync.dma_start(
                                out=L["rows"][c0:c0 + gw4, :].rearrange(
                                    "n f -> f n"
                                ),
                                in_=osl,
                            )
                        else:
                            ot = otp.tile([fout, TG * 128], BF16, tag="ot")
                            nc.scalar.activation(
                                ot[:, :gw4], pb[:, :gw4],
                                mybir.ActivationFunctionType.Identity,
                                bias=L["b"][:],
                            )
                            nc.sync.dma_start(
                                out=out_d[:, c0:c0 + gw4], in_=ot[:, :gw4]
                            )
            if L["tab_next"] is not None:
                nc.gpsimd.collective_compute(
                    "AllGather",
                    mybir.AluOpType.bypass,
                    replica_groups=rg,
                    ins=[L["rows"][:, :]],
                    outs=[L["tab_next"][:, :]],
                )

    nc.compile()
    return nc


# ------------------------------------------------------------------ runner
def make_in_maps(inputs, n_nodes, npc, npcp, fin, fhid, fout, per_core):
    n_tiles = npcp // 128
    t_chunks = per_core[0]["packed"].shape[1]
    t2 = t_chunks + (t_chunks & 1)
    x = np.asarray(inputs["x"], dtype=np.float32)

    consts = np.zeros((128, CW), dtype=NP_BF16)
    consts[:, IOTA_C:IOTA_C + 128] = np.arange(128, dtype=np.float32)[
        None, :
    ].astype(NP_BF16)
    for name, c0 in (("W1_0", W10_C), ("W1_1", W11_C), ("Wx_0", WX0_C),
                     ("Wx_1", WX1_C), ("W2_0", W20_C), ("W2_1", W21_C)):
        w = np.asarray(inputs[name], np.float32).astype(NP_BF16)
        consts[0:w.shape[0], c0:c0 + w.shape[1]] = w
    consts[0:fhid, B1_C] = np.asarray(inputs["b1"], np.float32).astype(
        NP_BF16
    )
    consts[0:fhid, BX_C] = np.asarray(inputs["bx"], np.float32).astype(
        NP_BF16
    )
    consts[0:fout, B2_C] = np.asarray(inputs["b2"], np.float32).astype(
        NP_BF16
    )

    in_maps = []
    for c in range(M_CORES):
        xp = np.zeros((npcp, 2 * XW), dtype=np.float32)
        xp[:npc, :fin] = x[c * npc:(c + 1) * npc]
        xtiles = np.ascontiguousarray(
            xp.reshape(n_tiles, 128, 2 * XW).transpose(1, 0, 2).reshape(
                128, n_tiles * 2 * XW
            )
        ).astype(NP_BF16)
        wpad = np.zeros((128, t2), dtype=NP_BF16)
        wpad[:, :t_chunks] = per_core[c]["wgt"].astype(NP_BF16)
        payload = np.concatenate([xtiles, consts, wpad], axis=1)
        blob = np.concatenate(
            [per_core[c]["packed"], payload.view(np.int32)], axis=1
        )
        in_maps.append(dict(blob=np.ascontiguousarray(blob)))
    return in_maps


def run(inputs, n_nodes, fin, fhid, fout, trace=False, trace_kwargs=None,
        timeit=0):
    npc = n_nodes // M_CORES
    npcp = int(math.ceil(npc / 128.0)) * 128

    adj = np.asarray(inputs["adj"], dtype=np.int32)
    sched, per_core = host_prep(adj, n_nodes, npc, npcp)
    nc = build_program(sched, npcp, fin, fhid, fout)
    in_maps = make_in_maps(
        inputs, n_nodes, npc, npcp, fin, fhid, fout, per_core
    )
    res = run_bass_kernel_spmd(
        nc,
        in_maps,
        core_ids=list(range(M_CORES)),
        trace=trace,
        **(trace_kwargs or {}),
    )
    times = []
    for _ in range(timeit):
        t0 = time.perf_counter()
        run_bass_kernel_spmd(nc, in_maps, core_ids=list(range(M_CORES)))
        times.append(time.perf_counter() - t0)
    if times:
        print("repeat wall times (s):", [f"{t:.3f}" for t in times])
        global LAST_TIMES
        LAST_TIMES = times
    out = np.concatenate(
        [
            np.asarray(res.results[c]["out"])[:, :npc].T.astype(np.float32)
            for c in range(M_CORES)
        ],
        axis=0,
    )
    return out, res


def kernel(**inputs):
    out, _ = run(inputs, n_nodes=100000, fin=64, fhid=64, fout=16)
    return out


# revision 12
# speedup vs baseline: 15.3861x; 1.2931x over previous
"""ChebGCN (K=2, 3 layers) Trainium2 kernel — 8-core SPMD.

Sharding: nodes are split across 8 cores (12500/core). Within a core,
local nodes are PERMUTED so they are grouped by degree bucket (host-side
renumbering; inputs/outputs are permuted on the host for free). Each
node's incoming edges occupy a fixed per-bucket budget of M slots
(M in {12,16,20,24,tail}, zero-padded), so the whole propagate becomes:
indirect-DMA gather of 128 source rows per (tile, occurrence) slot column,
one broadcast tensor_tensor to apply edge weights, and ONE tensor_reduce
over the innermost occurrence axis per tile group — no per-chunk matmuls.
Per layer the reduced segment sums are stored row-major by transposed-AP
DMA, re-transposed whole-table by one DMA-transpose for the dense term,
and the dense 64-wide weight matmuls + bias/relu run 4 node-tiles (512
cols) per op in transposed layout with running features resident in SBUF.
An 8-core AllGather rebuilds the global row table per layer.

Host->device traffic is one int32 tensor per core packing, bit-cast per
region: per-slot source indices (int32) and weights (bf16), the permuted
bf16 x slice, and weight/bias constants. Output leaves as bf16 and is
un-permuted on the host.
"""

import sys

for _p in ("/opt/trn_rl_repo",):
    if _p not in sys.path:
        sys.path.insert(0, _p)

import math
import time
from contextlib import ExitStack

import ml_dtypes
import numpy as np

import concourse.bacc as bacc
import concourse.bass as bass
import concourse.mybir as mybir
import concourse.tile as tile
from concourse.bass_utils import run_bass_kernel_spmd

F32 = mybir.dt.float32
I32 = mybir.dt.int32
BF16 = mybir.dt.bfloat16
NP_BF16 = ml_dtypes.bfloat16

M_CORES = 8
TG = 4        # node tiles per dense matmul batch (512 cols)
XW = 32       # int32 cols per x tile in the blob (64 bf16 feats)
TMAX = 96     # max slot-columns (Tt*M) per gather/reduce group
CW = 292      # bf16 const-region cols (even)
W10_C, W11_C, WX0_C, WX1_C = 0, 64, 128, 192
W20_C, W21_C, B1_C, BX_C, B2_C = 256, 272, 288, 289, 290
LAST_TIMES = []  # wall times of repeat runs (filled by run(timeit=N))


# ---------------------------------------------------------------- host prep
def host_prep(adj, n_nodes, npc):
    """Degree-bucket nodes per core, build slot tables.

    Returns dict with: groups [(t0, Tt, M, col0)], tile_m, n_tiles2, tw,
    per_core [{offs [128,TW] i32, wgt [128,TW] f32}], new_of_old [8][npc].
    """
    row = adj[0].astype(np.int64)
    col = adj[1].astype(np.int64)
    deg = np.bincount(row, minlength=n_nodes).astype(np.int64)
    dis = np.where(deg > 0, 1.0 / np.sqrt(np.maximum(deg, 1)), 0.0).astype(
        np.float32
    )
    w_all = (-(dis[row] * dis[col])).astype(np.float32)

    maxdeg = int(deg.max())
    ms = [m for m in (12, 16, 20, 24) if m < maxdeg]
    ms.append(max(maxdeg, (ms[-1] + 4) if ms else 12))
    nb = len(ms)
    ms_arr = np.array(ms)

    def bucket_of(d):
        return np.searchsorted(ms_arr, np.maximum(d, 1))

    b_of = [bucket_of(deg[c * npc:(c + 1) * npc]) for c in range(M_CORES)]
    counts = np.zeros((M_CORES, nb), dtype=np.int64)
    for c in range(M_CORES):
        counts[c] = np.bincount(b_of[c], minlength=nb)
    nb_tiles = np.array(
        [int(math.ceil(counts[:, b].max() / 128.0)) for b in range(nb)]
    )
    n_tiles2 = int(nb_tiles.sum())
    npcp2 = 128 * n_tiles2

    tile_m = []
    for b in range(nb):
        tile_m += [ms[b]] * int(nb_tiles[b])
    col0 = np.concatenate([[0], np.cumsum(tile_m)[:-1]]).astype(np.int64)
    tw = int(np.sum(tile_m))

    groups = []
    t = 0
    while t < n_tiles2:
        m = tile_m[t]
        tt = max(1, TMAX // m)
        tt = min(tt, n_tiles2 - t)
        while tile_m[t + tt - 1] != m:
            tt -= 1
        groups.append((t, tt, m, int(col0[t])))
        t += tt

    base_b = 128 * np.concatenate([[0], np.cumsum(nb_tiles)[:-1]])
    new_of_old = []
    for c in range(M_CORES):
        order = np.argsort(b_of[c], kind="stable")
        noo = np.zeros(npc, dtype=np.int64)
        pos_in_b = np.zeros(nb, dtype=np.int64)
        sorted_b = b_of[c][order]
        # positions within each bucket follow sorted order
        starts = np.searchsorted(sorted_b, np.arange(nb))
        noo[order] = base_b[sorted_b] + (np.arange(npc) - starts[sorted_b])
        new_of_old.append(noo)

    # global padded source index per edge
    sc = col // npc
    sl = col % npc
    noo_all = np.stack(new_of_old)  # [8, npc]
    colp = sc * npcp2 + noo_all[sc, sl]

    per_core = []
    core_of = row // npc
    for c in range(M_CORES):
        sel = np.nonzero(core_of == c)[0]
        d_loc = row[sel] - c * npc
        q = new_of_old[c][d_loc]
        order = np.argsort(q, kind="stable")
        sel = sel[order]
        qs = q[order]
        # occurrence j within each destination's slot budget
        seg_start = np.searchsorted(qs, qs)
        j = np.arange(len(qs)) - seg_start
        t_of = qs // 128
        p_of = qs % 128
        cols = col0[t_of] + j
        offs = np.zeros((128, tw), dtype=np.int32)
        wgt = np.zeros((128, tw), dtype=np.float32)
        offs[p_of, cols] = colp[sel]
        wgt[p_of, cols] = w_all[sel]
        per_core.append(dict(offs=offs, wgt=wgt))

    return dict(groups=groups, tile_m=tile_m, n_tiles2=n_tiles2, tw=tw,
                per_core=per_core, new_of_old=new_of_old, npcp2=npcp2)


def blob_geom(tw, n_tiles2):
    tw2 = tw + (tw & 1)
    xb = tw                      # x region start (i32 cols)
    cb = xb + n_tiles2 * XW      # const region
    wb = cb + CW // 2            # weight region
    w32 = wb + tw2 // 2
    return xb, cb, wb, w32


# ------------------------------------------------------------- bass program
def build_program(hp, fin, fhid, fout):
    groups = hp["groups"]
    n_tiles2 = hp["n_tiles2"]
    tw = hp["tw"]
    npcp2 = hp["npcp2"]
    np_all = npcp2 * M_CORES
    tw2 = tw + (tw & 1)
    xb, cbase, wbase, w32 = blob_geom(tw, n_tiles2)

    nc = bacc.Bacc(
        "TRN2",
        target_bir_lowering=False,
        debug=False,
        enable_asserts=False,
        num_devices=M_CORES,
    )

    blob_d = nc.dram_tensor("blob", [128, w32], I32, kind="ExternalInput")
    out_d = nc.dram_tensor("out", [fout, npcp2], BF16, kind="ExternalOutput")

    xrows = nc.dram_tensor("xrows", [npcp2, fin], BF16)
    rows1 = nc.dram_tensor("rows1", [npcp2, fhid], BF16)
    rows2 = nc.dram_tensor("rows2", [npcp2, fhid], BF16)
    tx1_d = nc.dram_tensor("tx1", [npcp2, fhid], BF16)
    tx2_d = nc.dram_tensor("tx2", [npcp2, fhid], BF16)
    tx3_d = nc.dram_tensor("tx3", [npcp2, fhid], BF16)
    tab1 = nc.dram_tensor("tab1", [np_all, fin], BF16, addr_space="Shared")
    tab2 = nc.dram_tensor("tab2", [np_all, fhid], BF16, addr_space="Shared")
    tab3 = nc.dram_tensor("tab3", [np_all, fhid], BF16, addr_space="Shared")

    rg = [list(range(M_CORES))]

    with ExitStack() as ctx:
        tc = ctx.enter_context(tile.TileContext(nc))
        const = ctx.enter_context(tc.tile_pool(name="const", bufs=1))
        rp = ctx.enter_context(tc.tile_pool(name="rp", bufs=2))
        rwp = ctx.enter_context(tc.tile_pool(name="rwp", bufs=2))
        txop = ctx.enter_context(tc.tile_pool(name="txop", bufs=2))
        otp = ctx.enter_context(tc.tile_pool(name="otp", bufs=2))
        psB = ctx.enter_context(tc.tile_pool(name="psB", bufs=2, space="PSUM"))

        # const region: one DMA, then slice views
        cb = const.tile([128, CW], BF16, tag="cb")
        nc.sync.dma_start(
            out=cb[:],
            in_=blob_d[:, cbase:cbase + CW // 2].bitcast(BF16),
        )
        w10_t = cb[0:fin, W10_C:W10_C + fhid]
        w11_t = cb[0:fin, W11_C:W11_C + fhid]
        wx0_t = cb[0:fhid, WX0_C:WX0_C + fhid]
        wx1_t = cb[0:fhid, WX1_C:WX1_C + fhid]
        w20_t = cb[0:fhid, W20_C:W20_C + fout]
        w21_t = cb[0:fhid, W21_C:W21_C + fout]
        b1_t = const.tile([fhid, 1], F32, tag="b1")
        nc.vector.tensor_copy(out=b1_t[:], in_=cb[0:fhid, B1_C:B1_C + 1])
        bx_t = const.tile([fhid, 1], F32, tag="bx")
        nc.vector.tensor_copy(out=bx_t[:], in_=cb[0:fhid, BX_C:BX_C + 1])
        b2_t = const.tile([fout, 1], F32, tag="b2")
        nc.vector.tensor_copy(out=b2_t[:], in_=cb[0:fout, B2_C:B2_C + 1])

        # slot metadata: two bulk DMAs, no unpacking needed
        offs_all = const.tile([128, tw], I32, tag="offs")
        nc.sync.dma_start(out=offs_all[:], in_=blob_d[:, 0:tw])
        w_sb = const.tile([128, tw2], BF16, tag="wal")
        nc.sync.dma_start(
            out=w_sb[:],
            in_=blob_d[:, wbase:wbase + tw2 // 2].bitcast(BF16),
        )

        # x prologue: bulk load, row-major store, AllGather, transpose
        xa = const.tile([128, n_tiles2 * XW], I32, tag="xa")
        nc.sync.dma_start(out=xa[:],
                          in_=blob_d[:, xb:xb + n_tiles2 * XW])
        xa16 = xa[:].bitcast(BF16).rearrange("p (t f) -> p t f", f=2 * XW)
        nc.sync.dma_start(
            out=xrows[:, :].rearrange("(t p) f -> p t f", p=128),
            in_=xa16,
        )
        nc.gpsimd.collective_compute(
            "AllGather",
            mybir.AluOpType.bypass,
            replica_groups=rg,
            ins=[xrows[:, :]],
            outs=[tab1[:, :]],
        )
        xT_sb = const.tile([fin, npcp2], BF16, tag="xT")
        nc.sync.dma_start_transpose(xT_sb[:], xrows[:, :])

        txT_sb = const.tile([fhid, npcp2], BF16, tag="txT")
        hT1_sb = const.tile([fhid, npcp2], BF16, tag="hT1")
        hT2_sb = const.tile([fhid, npcp2], BF16, tag="hT2")

        layers = [
            dict(table=tab1, rhs_sb=xT_sb, W0=w10_t, W1=w11_t, b=b1_t,
                 fo=fhid, hT_next=hT1_sb, rows=rows1, tab_next=tab2,
                 tx_d=tx1_d),
            dict(table=tab2, rhs_sb=hT1_sb, W0=wx0_t, W1=wx1_t, b=bx_t,
                 fo=fhid, hT_next=hT2_sb, rows=rows2, tab_next=tab3,
                 tx_d=tx2_d),
            dict(table=tab3, rhs_sb=hT2_sb, W0=w20_t, W1=w21_t, b=b2_t,
                 fo=fout, hT_next=None, rows=None, tab_next=None,
                 tx_d=tx3_d),
        ]

        for li, L in enumerate(layers):
            fo = L["fo"]
            # propagate: gather / weight / reduce per tile group
            for (t0, tt, m, c0) in groups:
                r = rp.tile([128, tt, m, fin], BF16, tag="r")
                for t in range(tt):
                    for j in range(m):
                        nc.gpsimd.indirect_dma_start(
                            out=r[:, t, j, :],
                            out_offset=None,
                            in_=L["table"][:, :],
                            in_offset=bass.IndirectOffsetOnAxis(
                                ap=offs_all[:, c0 + t * m + j:
                                            c0 + t * m + j + 1],
                                axis=0,
                            ),
                        )
                rw = rwp.tile([128, tt, m, fin], BF16, tag="rw")
                nc.vector.tensor_tensor(
                    out=rw[:],
                    in0=r[:],
                    in1=w_sb[:, c0:c0 + tt * m].rearrange(
                        "p (t m) -> p t m", m=m
                    ).unsqueeze(3).broadcast_to([128, tt, m, fin]),
                    op=mybir.AluOpType.mult,
                )
                txo = txop.tile([128, tt, fin], BF16, tag="txo")
                with nc.allow_low_precision(
                    reason="segment sums of <=40 bf16 terms; tolerance 2e-2"
                ):
                    nc.vector.tensor_reduce(
                        out=txo[:],
                        in_=rw[:].rearrange("p t m f -> p t f m"),
                        axis=mybir.AxisListType.X,
                        op=mybir.AluOpType.add,
                    )
                nc.sync.dma_start(
                    out=L["tx_d"][t0 * 128:(t0 + tt) * 128, :].rearrange(
                        "(t p) f -> p t f", p=128
                    ),
                    in_=txo[:],
                )
            # whole-layer transpose of segment sums for the dense term
            nc.sync.dma_start_transpose(txT_sb[:], L["tx_d"][:, :])
            # dense + bias/relu, TG node tiles per op
            for q0 in range(0, n_tiles2, TG):
                qt = min(TG, n_tiles2 - q0)
                c0 = q0 * 128
                gw = qt * 128
                pb = psB.tile([fo, TG * 128], F32, tag="pb")
                nc.tensor.matmul(
                    pb[:, :gw], lhsT=L["W0"],
                    rhs=L["rhs_sb"][:, c0:c0 + gw],
                    start=True, stop=False,
                )
                nc.tensor.matmul(
                    pb[:, :gw], lhsT=L["W1"],
                    rhs=txT_sb[:, c0:c0 + gw],
                    start=False, stop=True,
                )
                if L["hT_next"] is not None:
                    osl = L["hT_next"][:, c0:c0 + gw]
                    nc.scalar.activation(
                        osl, pb[:, :gw],
                        mybir.ActivationFunctionType.Relu,
                        bias=L["b"][:],
                    )
                    nc.sync.dma_start(
                        out=L["rows"][c0:c0 + gw, :].rearrange(
                            "n f -> f n"
                        ),
                        in_=osl,
                    )
                else:
                    ot = otp.tile([fout, TG * 128], BF16, tag="ot")
                    nc.scalar.activation(
                        ot[:, :gw], pb[:, :gw],
                        mybir.ActivationFunctionType.Identity,
                        bias=L["b"][:],
                    )
                    nc.sync.dma_start(
                        out=out_d[:, c0:c0 + gw], in_=ot[:, :gw]
                    )
            if L["tab_next"] is not None:
                nc.gpsimd.collective_compute(
                    "AllGather",
                    mybir.AluOpType.bypass,
                    replica_groups=rg,
                    ins=[L["rows"][:, :]],
                    outs=[L["tab_next"][:, :]],
                )

    nc.compile()
    return nc


# ------------------------------------------------------------------ runner
def make_in_maps(inputs, n_nodes, npc, hp, fin, fhid, fout):
    n_tiles2 = hp["n_tiles2"]
    npcp2 = hp["npcp2"]
    tw = hp["tw"]
    tw2 = tw + (tw & 1)
    x = np.asarray(inputs["x"], dtype=np.float32)

    consts = np.zeros((128, CW), dtype=NP_BF16)
    for name, c0 in (("W1_0", W10_C), ("W1_1", W11_C), ("Wx_0", WX0_C),
                     ("Wx_1", WX1_C), ("W2_0", W20_C), ("W2_1", W21_C)):
        w = np.asarray(inputs[name], np.float32).astype(NP_BF16)
        consts[0:w.shape[0], c0:c0 + w.shape[1]] = w
    consts[0:fhid, B1_C] = np.asarray(inputs["b1"], np.float32).astype(
        NP_BF16
    )
    consts[0:fhid, BX_C] = np.asarray(inputs["bx"], np.float32).astype(
        NP_BF16
    )
    consts[0:fout, B2_C] = np.asarray(inputs["b2"], np.float32).astype(
        NP_BF16
    )

    in_maps = []
    for c in range(M_CORES):
        xp = np.zeros((npcp2, 2 * XW), dtype=np.float32)
        xp[hp["new_of_old"][c], :fin] = x[c * npc:(c + 1) * npc]
        xtiles = np.ascontiguousarray(
            xp.reshape(n_tiles2, 128, 2 * XW).transpose(1, 0, 2).reshape(
                128, n_tiles2 * 2 * XW
            )
        ).astype(NP_BF16)
        wpad = np.zeros((128, tw2), dtype=NP_BF16)
        wpad[:, :tw] = hp["per_core"][c]["wgt"].astype(NP_BF16)
        payload = np.concatenate([xtiles, consts, wpad], axis=1)
        blob = np.concatenate(
            [hp["per_core"][c]["offs"], payload.view(np.int32)], axis=1
        )
        in_maps.append(dict(blob=np.ascontiguousarray(blob)))
    return in_maps


def run(inputs, n_nodes, fin, fhid, fout, trace=False, trace_kwargs=None,
        timeit=0):
    npc = n_nodes // M_CORES

    adj = np.asarray(inputs["adj"], dtype=np.int32)
    hp = host_prep(adj, n_nodes, npc)
    nc = build_program(hp, fin, fhid, fout)
    in_maps = make_in_maps(inputs, n_nodes, npc, hp, fin, fhid, fout)
    res = run_bass_kernel_spmd(
        nc,
        in_maps,
        core_ids=list(range(M_CORES)),
        trace=trace,
        **(trace_kwargs or {}),
    )
    times = []
    for _ in range(timeit):
        t0 = time.perf_counter()
        run_bass_kernel_spmd(nc, in_maps, core_ids=list(range(M_CORES)))
        times.append(time.perf_counter() - t0)
    if times:
        print("repeat wall times (s):", [f"{t:.3f}" for t in times])
        global LAST_TIMES
        LAST_TIMES = times
    out = np.concatenate(
        [
            np.asarray(res.results[c]["out"])[:, hp["new_of_old"][c]]
            .T.astype(np.float32)
            for c in range(M_CORES)
        ],
        axis=0,
    )
    return out, res


def kernel(**inputs):
    out, _ = run(inputs, n_nodes=100000, fin=64, fhid=64, fout=16)
    return out
